# revision 2
# baseline (speedup 1.0000x reference)
"""Trainium2 Bass kernel for cosine-similarity contrastive loss (CosSimLoss).

reference:
    p = l2norm(pred).reshape(-1, C); t = l2norm(target).reshape(-1, C)
    logits = (p @ t.T) * e^0.5
    loss = mean(logsumexp(logits, axis=1) - diag(logits))

Strategy (8 NeuronCores, data parallel over N = B*L = 8192 rows of pred):
  Host converts inputs to bf16 (pure layout/dtype prep) so each core loads
  the full target in half the bytes (8MB).  Per block of target rows the
  core computes row norms (DVE bf16 square-accum), rsqrts them with an
  Ln+Exp pair on the Act engine (a doctored act-table list pins the single
  natural_log_exp_and_others set, so Exp and Ln never swap tables), casts
  to fp8e4 on the Pool engine with the row-pair interleaved write, bounces
  to DRAM on the gpsimd DMA queue, and u16-xbar transposes to planar
  transposed fp8 on the sync queue.  pred (bf16 input) is xbar-transposed
  straight from its DRAM input tensor and cast to planar fp8 weights on
  Pool.  fp8 DoubleRow matmuls contract K=256 per instruction; the Exp
  activation runs in-place on PSUM with the 1/||p|| factor folded into its
  per-partition scale and a fused row-sum accumulator.  The diagonal is
  computed exactly in bf16/fp32 from the pred shard and the matching
  target shard (td input keeps the program SPMD).  Host sums the per-core
  (lse - diag) partials and divides by N.
"""
import math

import numpy as np

import concourse.bacc as bacc
import concourse.mybir as mybir
import concourse.tile as tile
from concourse.bass_utils import run_bass_kernel_spmd
from concourse.hw_specs import get_activation_tables

F32 = mybir.dt.float32
BF16 = mybir.dt.bfloat16
FP8 = mybir.dt.float8e4
U16 = mybir.dt.uint16
AF = mybir.ActivationFunctionType
ALU = mybir.AluOpType
AXIS = mybir.AxisListType
PM = mybir.MatmulPerfMode

TEMPERATURE = 0.5
SCALE = float(math.exp(TEMPERATURE))
FP8_GAIN = 8.0  # normalized target rows scaled by this before fp8 cast

# Full problem config (hardcoded per contest rules).
B, L, C = 4, 2048, 512
N_CORES = 8
N_TOTAL = B * L                  # 8192
M_LOCAL = N_TOTAL // N_CORES     # 1024 rows per core
MT = M_LOCAL // 128              # 8 output row tiles
KQ = C // 256                    # 2 fp8-pair chunks of the contraction

# Ramped target blocks: small first block collapses startup latency.
BLOCKS = [(0, 512), (512, 1536), (2048, 2048), (4096, 2048), (6144, 2048)]
NB = len(BLOCKS)
BLK = 2048                       # max block size (psum/t8b padding)


class OneSetBacc(bacc.Bacc):
    """Bacc whose act-table chooser only ever sees one non-empty set.

    natural_log_exp_and_others contains Exp, Ln, Square and Copy — every
    activation this kernel uses — so the table is loaded once and never
    swapped.  Indices stay aligned with act_info.json (other sets are
    emptied, not removed), so the emitted act_func_set_id stays valid.
    """

    def insert_act_table_loads(self):
        has_activation = any(
            isinstance(i, mybir.InstActivation)
            for b in self.main_func.blocks
            for i in b.instructions
        )
        if not has_activation:
            return
        tables = []
        for name, funcs in get_activation_tables(self.m.arch).items():
            tables.append(
                (name, funcs if name == "natural_log_exp_and_others" else set())
            )
        bacc._bass_rust.insert_act_table_loads(self, tables)


def build_nc():
    """Build + compile the per-core Bass program (SPMD: same NEFF, 8 cores)."""
    nc = OneSetBacc("TRN2", target_bir_lowering=False, debug=False)
    pred = nc.dram_tensor("pred", [M_LOCAL, C], BF16, kind="ExternalInput").ap()
    tgt = nc.dram_tensor("tgt", [N_TOTAL, C], BF16, kind="ExternalInput").ap()
    td = nc.dram_tensor("td", [M_LOCAL, C], BF16, kind="ExternalInput").ap()
    out = nc.dram_tensor("out", [128, MT], F32, kind="ExternalOutput").ap()

    with tile.TileContext(nc) as tc:
        with (
            tc.tile_pool(name="dram", bufs=1, space="DRAM") as dram_pool,
            tc.tile_pool(name="ld", bufs=1) as ld_pool,
            tc.tile_pool(name="tq", bufs=1) as tq_pool,
            tc.tile_pool(name="q8", bufs=1) as q8_pool,
            tc.tile_pool(name="sq", bufs=2) as sq_pool,
            tc.tile_pool(name="st", bufs=1) as stats_pool,
            tc.tile_pool(name="pT", bufs=1) as pT_pool,
            tc.tile_pool(name="tT", bufs=1) as tT_pool,
            tc.tile_pool(name="ps", bufs=2, space="PSUM") as psum_pool,
        ):
            # Target fp8 bounce, row-PAIR interleaved bytes: DRAM np-row
            # np = n//2 holds byte d = 2c + (n%2).  The u16 xbar transpose
            # of that layout yields PLANAR transposed fp8.
            t8_dram = dram_pool.tile([N_TOTAL // 2, 2 * C], FP8, name="t8d",
                                     tag="t8d")
            t8u = t8_dram.bitcast(U16)  # [N/2, 512] u16; col u = channel u
            sume = stats_pool.tile([128, MT * NB], F32, name="sume",
                                   tag="sume")

            tq_tiles = {}

            def load_block(g):
                # one DMA per block: partition p holds rows goff+s*p..+s-1
                goff, bsz = BLOCKS[g]
                s = bsz // 128
                t_ld = tq_pool.tile([128, s * C], BF16, name=f"tq{g}",
                                    tag=f"tq{g}", padded_shape=[128, 16 * C])
                nc.gpsimd.dma_start(
                    t_ld[:].rearrange("p (s c) -> p s c", c=C),
                    tgt[goff:goff + bsz, :].rearrange("(p s) c -> p s c", s=s))
                tq_tiles[g] = t_ld

            def block_norms(g):
                goff, bsz = BLOCKS[g]
                s = bsz // 128
                tqd = tq_tiles[g]
                stt = stats_pool.tile([128, s], F32, name=f"stt{g}",
                                      tag=f"stt{g}")
                for si in range(s):
                    a = tqd[:, si * C:(si + 1) * C]
                    sqd = sq_pool.tile([128, C], BF16, name="sqd", tag="sqd")
                    nc.vector.scalar_tensor_tensor(
                        sqd[:], a, 1.0, a, ALU.mult, ALU.mult,
                        accum_out=stt[:, si:si + 1])
                return stt

            def rsqrt(key, stt, cols):
                # single act-table set: Ln+Exp cost ~0.5us, no table swap
                ltt = stats_pool.tile([128, cols], F32, name=f"ltt{key}",
                                      tag=f"ltt{key}")
                nc.scalar.activation(ltt[:], stt[:], AF.Ln)
                rtt = stats_pool.tile([128, cols], F32, name=f"rtt{key}",
                                      tag=f"rtt{key}")
                nc.scalar.activation(rtt[:], ltt[:], AF.Exp, scale=-0.5)
                return rtt

            def block_cast_bounce(g, rtt):
                # fp8 cast with row-pair interleaved write (Pool) + one
                # contiguous bounce DMA (gpsimd queue)
                goff, bsz = BLOCKS[g]
                s = bsz // 128
                tqd = tq_tiles[g]
                t8b = q8_pool.tile([128, s * C], FP8, name="t8b", tag="t8b",
                                   bufs=3, padded_shape=[128, 16 * C])
                for si in range(s):
                    a = si // 2
                    seg = t8b[:, a * 1024:(a + 1) * 1024]
                    t8_out = seg.rearrange(
                        "p (c two) -> p two c",
                        two=2)[:, si % 2:si % 2 + 1, :].rearrange(
                            "p a c -> p (a c)")
                    nc.gpsimd.tensor_scalar(
                        t8_out, tqd[:, si * C:(si + 1) * C],
                        rtt[:, si:si + 1], FP8_GAIN,
                        ALU.mult, ALU.mult)
                nc.gpsimd.dma_start(
                    t8_dram[goff // 2:(goff + bsz) // 2, :].rearrange(
                        "(p t) d -> p t d", t=s // 2),
                    t8b[:, :s * C].rearrange("p (t d) -> p t d", d=2 * C))

            def block_transpose(g):
                # planar fp8 transposed tiles: tt[q] holds channel planes
                # (2q, 2q+1); plane i partition j = channel 256q + 128i + j
                goff, bsz = BLOCKS[g]
                tTg = []
                for q in range(KQ):
                    tt = tT_pool.tile([128, 2 * bsz], FP8, name="tT",
                                      tag="tT", bufs=4,
                                      padded_shape=[128, 2 * BLK])
                    ttu = tt.bitcast(U16).rearrange("p (i n) -> p i n", i=2)
                    for i in range(2):
                        qt = 2 * q + i
                        nc.sync.dma_start_transpose(
                            ttu[:, i:i + 1, :],
                            t8u[goff // 2:(goff + bsz) // 2,
                                qt * 128:(qt + 1) * 128])
                    tTg.append(tt)
                return tTg

            def block_matmul(g, tTg):
                goff, bsz = BLOCKS[g]
                for m in range(MT):
                    ps = psum_pool.tile([128, bsz], F32, name="ps", tag="ps",
                                        padded_shape=[128, BLK])
                    for q in range(KQ):
                        w_ap = pw[q].rearrange(
                            "j (i m) -> j i m",
                            i=2)[:, :, 128 * m:128 * (m + 1)]
                        x3 = tTg[q].rearrange("j (i n) -> j i n", i=2)
                        for j in range(bsz // 512):
                            nc.tensor.matmul(
                                ps[:, j * 512:(j + 1) * 512], w_ap,
                                x3[:, :, j * 512:(j + 1) * 512],
                                start=(q == 0), stop=(q == KQ - 1),
                                perf_mode=PM.DoubleRow)
                    nc.scalar.activation(
                        ps[:], ps[:], AF.Exp, scale=expsc[:, m:m + 1],
                        accum_out=sume[:, m * NB + g:m * NB + g + 1])

            # ---------------- loads (issue early, transfers pipeline) ------
            load_block(0)
            # pred/td in m-tile-major layout: partition p col q = row 128q+p
            pq = ld_pool.tile([128, MT * C], BF16, name="pld", tag="pld")
            nc.sync.dma_start(pq[:].rearrange("p (q c) -> p q c", c=C),
                              pred[:].rearrange("(q p) c -> p q c", p=128))
            tdq = ld_pool.tile([128, MT * C], BF16, name="tdld", tag="tdld")
            nc.sync.dma_start(tdq[:].rearrange("p (q c) -> p q c", c=C),
                              td[:].rearrange("(q p) c -> p q c", p=128))
            load_block(1)
            load_block(2)
            load_block(3)
            load_block(4)

            # pred transposed planes straight from the DRAM input (bf16 is
            # 2-byte, so the u16 xbar transpose is exact)
            pTb = []
            for kc in range(C // 128):
                pt = pT_pool.tile([128, M_LOCAL], BF16, name=f"pTb{kc}",
                                  tag=f"pTb{kc}")
                nc.sync.dma_start_transpose(
                    pt[:], pred[0:M_LOCAL, kc * 128:(kc + 1) * 128])
                pTb.append(pt)
            # planar fp8 weights (Pool casts)
            pw = []
            for q in range(KQ):
                w = pT_pool.tile([128, 2 * M_LOCAL], FP8, name=f"pw{q}",
                                 tag=f"pw{q}")
                for i in range(2):
                    nc.gpsimd.tensor_scalar_mul(
                        w[:, i * M_LOCAL:(i + 1) * M_LOCAL],
                        pTb[2 * q + i][:], 1.0)
                pw.append(w)

            # block 0+1 norms and rsqrts; pred/td norms; exp scales
            stt0 = block_norms(0)
            rt0 = rsqrt(0, stt0, 4)
            block_cast_bounce(0, rt0)

            stt1 = block_norms(1)
            rt1 = rsqrt(1, stt1, 12)

            sp = stats_pool.tile([128, MT], F32, name="sp", tag="sp")
            std = stats_pool.tile([128, MT], F32, name="std", tag="std")
            for q in range(MT):
                a = pq[:, q * C:(q + 1) * C]
                b = tdq[:, q * C:(q + 1) * C]
                sqa = sq_pool.tile([128, C], BF16, name="sqa", tag="sqd")
                nc.vector.scalar_tensor_tensor(
                    sqa[:], a, 1.0, a, ALU.mult, ALU.mult,
                    accum_out=sp[:, q:q + 1])
                sqb = sq_pool.tile([128, C], BF16, name="sqb", tag="sqd")
                nc.vector.scalar_tensor_tensor(
                    sqb[:], b, 1.0, b, ALU.mult, ALU.mult,
                    accum_out=std[:, q:q + 1])
            rp = rsqrt("p", sp, MT)
            rtd = rsqrt("td", std, MT)

            expsc = stats_pool.tile([128, MT], F32, name="expsc", tag="expsc")
            nc.vector.tensor_scalar_mul(expsc[:], rp[:], SCALE / FP8_GAIN)

            block_cast_bounce(1, rt1)
            tT0 = block_transpose(0)

            # block 2 norms + rsqrt queued on Act BEFORE block 0's exps
            stt2 = block_norms(2)
            rt2 = rsqrt(2, stt2, 16)
            block_cast_bounce(2, rt2)

            block_matmul(0, tT0)
            tT1 = block_transpose(1)

            stt3 = block_norms(3)
            rt3 = rsqrt(3, stt3, 16)
            block_cast_bounce(3, rt3)

            block_matmul(1, tT1)
            tT2 = block_transpose(2)

            stt4 = block_norms(4)
            rt4 = rsqrt(4, stt4, 16)
            block_cast_bounce(4, rt4)

            block_matmul(2, tT2)
            tT3 = block_transpose(3)

            # diag dot products on DVE (idle window during matmuls)
            d0 = stats_pool.tile([128, MT], F32, name="d0", tag="d0")
            for q in range(MT):
                a = pq[:, q * C:(q + 1) * C]
                b = tdq[:, q * C:(q + 1) * C]
                sqc = sq_pool.tile([128, C], BF16, name="sqc", tag="sqd")
                nc.vector.scalar_tensor_tensor(
                    sqc[:], a, 1.0, b, ALU.mult, ALU.mult,
                    accum_out=d0[:, q:q + 1])
            dtmp = stats_pool.tile([128, MT], F32, name="dtmp", tag="dtmp")
            nc.vector.tensor_mul(dtmp[:], d0[:], rtd[:])
            diag = stats_pool.tile([128, MT], F32, name="diag", tag="diag")
            nc.vector.scalar_tensor_tensor(
                diag[:], dtmp[:], SCALE, rp[:], ALU.mult, ALU.mult)

            block_matmul(3, tT3)
            tT4 = block_transpose(4)
            block_matmul(4, tT4)

            # ---------------- lse - diag ----------------------------------
            rowsum = stats_pool.tile([128, MT], F32, name="rowsum",
                                     tag="rowsum")
            nc.vector.tensor_reduce(
                rowsum[:], sume[:].rearrange("p (m g) -> p m g", g=NB),
                axis=AXIS.X, op=ALU.add)
            lse = stats_pool.tile([128, MT], F32, name="lse", tag="lse")
            nc.scalar.activation(lse[:], rowsum[:], AF.Ln)
            losst = stats_pool.tile([128, MT], F32, name="losst", tag="losst")
            nc.vector.tensor_sub(losst[:], lse[:], diag[:])
            nc.sync.dma_start(out[:], losst[:])

    nc.compile()
    return nc


_NC_CACHE = {}


def _get_nc():
    key = (M_LOCAL, N_TOTAL, C)
    if key not in _NC_CACHE:
        _NC_CACHE[key] = build_nc()
    return _NC_CACHE[key]


def run_cores(pred2d, tgt2d, trace=False):
    """Run the SPMD program on cores 0..7; returns (partials [8,128,MT], res)."""
    import ml_dtypes
    bf16 = ml_dtypes.bfloat16
    nc = _get_nc()
    pred_b = np.asarray(pred2d, dtype=np.float32).astype(bf16)
    tgt_b = np.asarray(tgt2d, dtype=np.float32).astype(bf16)
    in_maps = []
    for ci in range(N_CORES):
        r0 = ci * M_LOCAL
        in_maps.append({
            "pred": np.ascontiguousarray(pred_b[r0:r0 + M_LOCAL]),
            "tgt": tgt_b,
            "td": np.ascontiguousarray(tgt_b[r0:r0 + M_LOCAL]),
        })
    res = run_bass_kernel_spmd(nc, in_maps, list(range(N_CORES)), trace=trace)
    partials = np.stack([res.results[i]["out"] for i in range(N_CORES)])
    return partials, res


def kernel(pred, target):
    pred2d = np.asarray(pred, dtype=np.float32).reshape(-1, C)
    tgt2d = np.asarray(target, dtype=np.float32).reshape(-1, C)
    partials, _ = run_cores(pred2d, tgt2d)
    loss = partials.astype(np.float64).sum() / float(N_TOTAL)
    return np.float32(loss)


# revision 3
# speedup vs baseline: 1.3456x; 1.3456x over previous
"""Trainium2 Bass kernel for cosine-similarity contrastive loss (CosSimLoss).

reference:
    p = l2norm(pred).reshape(-1, C); t = l2norm(target).reshape(-1, C)
    logits = (p @ t.T) * e^0.5
    loss = mean(logsumexp(logits, axis=1) - diag(logits))

Strategy (8 NeuronCores, data parallel over N = B*L = 8192 rows of pred):
  Host converts inputs to bf16 (pure layout/dtype prep) so each core loads
  the full target in half the bytes (8MB).  Per block of target rows the
  core computes row norms (DVE bf16 square-accum), rsqrts them with an
  Ln+Exp pair on the Act engine (a doctored act-table list pins the single
  natural_log_exp_and_others set, so Exp and Ln never swap tables), casts
  to fp8e4 on the Pool engine with the row-pair interleaved write, bounces
  to DRAM on the gpsimd DMA queue, and u16-xbar transposes to planar
  transposed fp8 on the sync queue.  pred (bf16 input) is xbar-transposed
  straight from its DRAM input tensor and cast to planar fp8 weights on
  Pool.  fp8 DoubleRow matmuls contract K=256 per instruction; the Exp
  activation runs in-place on PSUM with the 1/||p|| factor folded into its
  per-partition scale and a fused row-sum accumulator.  The diagonal is
  computed exactly in bf16/fp32 from the pred shard and the matching
  target shard (td input keeps the program SPMD).  Host sums the per-core
  (lse - diag) partials and divides by N.
"""
import math

import numpy as np

import concourse.bacc as bacc
import concourse.mybir as mybir
import concourse.tile as tile
from concourse.bass_utils import run_bass_kernel_spmd
from concourse.hw_specs import get_activation_tables

F32 = mybir.dt.float32
BF16 = mybir.dt.bfloat16
FP8 = mybir.dt.float8e4
U16 = mybir.dt.uint16
AF = mybir.ActivationFunctionType
ALU = mybir.AluOpType
AXIS = mybir.AxisListType
PM = mybir.MatmulPerfMode

TEMPERATURE = 0.5
SCALE = float(math.exp(TEMPERATURE))
FP8_GAIN = 8.0  # normalized target rows scaled by this before fp8 cast

# Full problem config (hardcoded per contest rules).
B, L, C = 4, 2048, 512
N_CORES = 8
N_TOTAL = B * L                  # 8192
M_LOCAL = N_TOTAL // N_CORES     # 1024 rows per core
MT = M_LOCAL // 128              # 8 output row tiles
KQ = C // 256                    # 2 fp8-pair chunks of the contraction

# Ramped target blocks: small first block collapses startup latency.
BLOCKS = [(0, 512), (512, 1536), (2048, 2048), (4096, 2048), (6144, 2048)]
NB = len(BLOCKS)
BLK = 2048                       # max block size (psum/t8b padding)


class OneSetBacc(bacc.Bacc):
    """Bacc whose act-table chooser only ever sees one non-empty set.

    natural_log_exp_and_others contains Exp, Ln, Square and Copy — every
    activation this kernel uses — so the table is loaded once and never
    swapped.  Indices stay aligned with act_info.json (other sets are
    emptied, not removed), so the emitted act_func_set_id stays valid.
    """

    def insert_act_table_loads(self):
        has_activation = any(
            isinstance(i, mybir.InstActivation)
            for b in self.main_func.blocks
            for i in b.instructions
        )
        if not has_activation:
            return
        tables = []
        for name, funcs in get_activation_tables(self.m.arch).items():
            tables.append(
                (name, funcs if name == "natural_log_exp_and_others" else set())
            )
        bacc._bass_rust.insert_act_table_loads(self, tables)


def build_nc():
    """Build + compile the per-core Bass program (SPMD: same NEFF, 8 cores)."""
    nc = OneSetBacc("TRN2", target_bir_lowering=False, debug=False)
    pred = nc.dram_tensor("pred", [M_LOCAL, C], BF16, kind="ExternalInput").ap()
    tgt = nc.dram_tensor("tgt", [N_TOTAL, C], BF16, kind="ExternalInput").ap()
    td = nc.dram_tensor("td", [M_LOCAL, C], BF16, kind="ExternalInput").ap()
    out = nc.dram_tensor("out", [128, MT], F32, kind="ExternalOutput").ap()

    with tile.TileContext(nc) as tc:
        with (
            tc.tile_pool(name="dram", bufs=1, space="DRAM") as dram_pool,
            tc.tile_pool(name="ld", bufs=1) as ld_pool,
            tc.tile_pool(name="tq", bufs=1) as tq_pool,
            tc.tile_pool(name="q8", bufs=1) as q8_pool,
            tc.tile_pool(name="sq", bufs=2) as sq_pool,
            tc.tile_pool(name="st", bufs=1) as stats_pool,
            tc.tile_pool(name="pT", bufs=1) as pT_pool,
            tc.tile_pool(name="tT", bufs=1) as tT_pool,
            tc.tile_pool(name="ps", bufs=2, space="PSUM") as psum_pool,
        ):
            # Target fp8 bounce, row-PAIR interleaved bytes: DRAM np-row
            # np = n//2 holds byte d = 2c + (n%2).  The u16 xbar transpose
            # of that layout yields PLANAR transposed fp8.
            t8_dram = dram_pool.tile([N_TOTAL // 2, 2 * C], FP8, name="t8d",
                                     tag="t8d")
            t8u = t8_dram.bitcast(U16)  # [N/2, 512] u16; col u = channel u
            sume = stats_pool.tile([128, MT * NB], F32, name="sume",
                                   tag="sume")

            tq_tiles = {}

            def load_block(g):
                # one DMA per block: partition p holds rows goff+s*p..+s-1
                goff, bsz = BLOCKS[g]
                s = bsz // 128
                t_ld = tq_pool.tile([128, s * C], BF16, name=f"tq{g}",
                                    tag=f"tq{g}", padded_shape=[128, 16 * C])
                nc.sync.dma_start(
                    t_ld[:].rearrange("p (s c) -> p s c", c=C),
                    tgt[goff:goff + bsz, :].rearrange("(p s) c -> p s c", s=s))
                tq_tiles[g] = t_ld

            def block_norms(g):
                goff, bsz = BLOCKS[g]
                s = bsz // 128
                tqd = tq_tiles[g]
                stt = stats_pool.tile([128, s], F32, name=f"stt{g}",
                                      tag=f"stt{g}")
                for si in range(s):
                    a = tqd[:, si * C:(si + 1) * C]
                    sqd = sq_pool.tile([128, C], BF16, name="sqd", tag="sqd")
                    nc.vector.scalar_tensor_tensor(
                        sqd[:], a, 1.0, a, ALU.mult, ALU.mult,
                        accum_out=stt[:, si:si + 1])
                return stt

            def rsqrt(key, stt, cols):
                # single act-table set: Ln+Exp cost ~0.5us, no table swap
                ltt = stats_pool.tile([128, cols], F32, name=f"ltt{key}",
                                      tag=f"ltt{key}")
                nc.scalar.activation(ltt[:], stt[:], AF.Ln)
                rtt = stats_pool.tile([128, cols], F32, name=f"rtt{key}",
                                      tag=f"rtt{key}")
                nc.scalar.activation(rtt[:], ltt[:], AF.Exp, scale=-0.5)
                return rtt

            def block_cast_bounce(g, rtt):
                # fp8 cast with row-pair interleaved write (Pool) + one
                # contiguous bounce DMA (gpsimd queue)
                goff, bsz = BLOCKS[g]
                s = bsz // 128
                tqd = tq_tiles[g]
                t8b = q8_pool.tile([128, s * C], FP8, name="t8b", tag="t8b",
                                   bufs=3, padded_shape=[128, 16 * C])
                for si in range(s):
                    a = si // 2
                    seg = t8b[:, a * 1024:(a + 1) * 1024]
                    t8_out = seg.rearrange(
                        "p (c two) -> p two c",
                        two=2)[:, si % 2:si % 2 + 1, :].rearrange(
                            "p a c -> p (a c)")
                    eng = nc.gpsimd if si % 4 == 3 else nc.vector
                    eng.tensor_scalar(
                        t8_out, tqd[:, si * C:(si + 1) * C],
                        rtt[:, si:si + 1], FP8_GAIN,
                        ALU.mult, ALU.mult)
                nc.gpsimd.dma_start(
                    t8_dram[goff // 2:(goff + bsz) // 2, :].rearrange(
                        "(p t) d -> p t d", t=s // 2),
                    t8b[:, :s * C].rearrange("p (t d) -> p t d", d=2 * C))

            def block_transpose(g):
                # planar fp8 transposed tiles: tt[q] holds channel planes
                # (2q, 2q+1); plane i partition j = channel 256q + 128i + j
                goff, bsz = BLOCKS[g]
                tTg = []
                for q in range(KQ):
                    tt = tT_pool.tile([128, 2 * bsz], FP8, name="tT",
                                      tag="tT", bufs=4,
                                      padded_shape=[128, 2 * BLK])
                    ttu = tt.bitcast(U16).rearrange("p (i n) -> p i n", i=2)
                    for i in range(2):
                        qt = 2 * q + i
                        nc.sync.dma_start_transpose(
                            ttu[:, i:i + 1, :],
                            t8u[goff // 2:(goff + bsz) // 2,
                                qt * 128:(qt + 1) * 128])
                    tTg.append(tt)
                return tTg

            def block_matmul(g, tTg):
                goff, bsz = BLOCKS[g]
                for m in range(MT):
                    ps = psum_pool.tile([128, bsz], F32, name="ps", tag="ps",
                                        padded_shape=[128, BLK])
                    for q in range(KQ):
                        w_ap = pw[q].rearrange(
                            "j (i m) -> j i m",
                            i=2)[:, :, 128 * m:128 * (m + 1)]
                        x3 = tTg[q].rearrange("j (i n) -> j i n", i=2)
                        for j in range(bsz // 512):
                            nc.tensor.matmul(
                                ps[:, j * 512:(j + 1) * 512], w_ap,
                                x3[:, :, j * 512:(j + 1) * 512],
                                start=(q == 0), stop=(q == KQ - 1),
                                perf_mode=PM.DoubleRow)
                    nc.scalar.activation(
                        ps[:], ps[:], AF.Exp, scale=expsc[:, m:m + 1],
                        accum_out=sume[:, m * NB + g:m * NB + g + 1])

            # ---------------- loads (issue early, transfers pipeline) ------
            load_block(0)
            # pred/td in m-tile-major layout: partition p col q = row 128q+p
            pq = ld_pool.tile([128, MT * C], BF16, name="pld", tag="pld")
            nc.scalar.dma_start(pq[:].rearrange("p (q c) -> p q c", c=C),
                              pred[:].rearrange("(q p) c -> p q c", p=128))
            tdq = ld_pool.tile([128, MT * C], BF16, name="tdld", tag="tdld")
            nc.scalar.dma_start(tdq[:].rearrange("p (q c) -> p q c", c=C),
                              td[:].rearrange("(q p) c -> p q c", p=128))
            load_block(1)
            load_block(2)
            load_block(3)
            load_block(4)

            # pred transposed planes straight from the DRAM input (bf16 is
            # 2-byte, so the u16 xbar transpose is exact)
            pTb = []
            for kc in range(C // 128):
                pt = pT_pool.tile([128, M_LOCAL], BF16, name=f"pTb{kc}",
                                  tag=f"pTb{kc}")
                nc.scalar.dma_start_transpose(
                    pt[:], pred[0:M_LOCAL, kc * 128:(kc + 1) * 128])
                pTb.append(pt)
            # planar fp8 weights (Pool casts)
            pw = []
            for q in range(KQ):
                w = pT_pool.tile([128, 2 * M_LOCAL], FP8, name=f"pw{q}",
                                 tag=f"pw{q}")
                for i in range(2):
                    nc.vector.tensor_scalar_mul(
                        w[:, i * M_LOCAL:(i + 1) * M_LOCAL],
                        pTb[2 * q + i][:], 1.0)
                pw.append(w)

            # block 0+1 norms and rsqrts; pred/td norms; exp scales
            stt0 = block_norms(0)
            rt0 = rsqrt(0, stt0, 4)
            block_cast_bounce(0, rt0)

            stt1 = block_norms(1)
            rt1 = rsqrt(1, stt1, 12)

            sp = stats_pool.tile([128, MT], F32, name="sp", tag="sp")
            std = stats_pool.tile([128, MT], F32, name="std", tag="std")
            for q in range(MT):
                a = pq[:, q * C:(q + 1) * C]
                b = tdq[:, q * C:(q + 1) * C]
                sqa = sq_pool.tile([128, C], BF16, name="sqa", tag="sqd")
                nc.vector.scalar_tensor_tensor(
                    sqa[:], a, 1.0, a, ALU.mult, ALU.mult,
                    accum_out=sp[:, q:q + 1])
                sqb = sq_pool.tile([128, C], BF16, name="sqb", tag="sqd")
                nc.vector.scalar_tensor_tensor(
                    sqb[:], b, 1.0, b, ALU.mult, ALU.mult,
                    accum_out=std[:, q:q + 1])
            rp = rsqrt("p", sp, MT)
            rtd = rsqrt("td", std, MT)

            expsc = stats_pool.tile([128, MT], F32, name="expsc", tag="expsc")
            nc.vector.tensor_scalar_mul(expsc[:], rp[:], SCALE / FP8_GAIN)

            block_cast_bounce(1, rt1)
            tT0 = block_transpose(0)

            # block 2 norms + rsqrt queued on Act BEFORE block 0's exps
            stt2 = block_norms(2)
            rt2 = rsqrt(2, stt2, 16)
            block_cast_bounce(2, rt2)

            block_matmul(0, tT0)
            tT1 = block_transpose(1)

            stt3 = block_norms(3)
            rt3 = rsqrt(3, stt3, 16)
            block_cast_bounce(3, rt3)

            block_matmul(1, tT1)
            tT2 = block_transpose(2)

            stt4 = block_norms(4)
            rt4 = rsqrt(4, stt4, 16)
            block_cast_bounce(4, rt4)

            block_matmul(2, tT2)
            tT3 = block_transpose(3)

            # diag dot products on DVE (idle window during matmuls)
            d0 = stats_pool.tile([128, MT], F32, name="d0", tag="d0")
            for q in range(MT):
                a = pq[:, q * C:(q + 1) * C]
                b = tdq[:, q * C:(q + 1) * C]
                sqc = sq_pool.tile([128, C], BF16, name="sqc", tag="sqd")
                nc.vector.scalar_tensor_tensor(
                    sqc[:], a, 1.0, b, ALU.mult, ALU.mult,
                    accum_out=d0[:, q:q + 1])
            dtmp = stats_pool.tile([128, MT], F32, name="dtmp", tag="dtmp")
            nc.vector.tensor_mul(dtmp[:], d0[:], rtd[:])
            diag = stats_pool.tile([128, MT], F32, name="diag", tag="diag")
            nc.vector.scalar_tensor_tensor(
                diag[:], dtmp[:], SCALE, rp[:], ALU.mult, ALU.mult)

            block_matmul(3, tT3)
            tT4 = block_transpose(4)
            block_matmul(4, tT4)

            # ---------------- lse - diag ----------------------------------
            rowsum = stats_pool.tile([128, MT], F32, name="rowsum",
                                     tag="rowsum")
            nc.vector.tensor_reduce(
                rowsum[:], sume[:].rearrange("p (m g) -> p m g", g=NB),
                axis=AXIS.X, op=ALU.add)
            lse = stats_pool.tile([128, MT], F32, name="lse", tag="lse")
            nc.scalar.activation(lse[:], rowsum[:], AF.Ln)
            losst = stats_pool.tile([128, MT], F32, name="losst", tag="losst")
            nc.vector.tensor_sub(losst[:], lse[:], diag[:])
            nc.scalar.dma_start(out[:], losst[:])

    nc.compile()
    return nc


_NC_CACHE = {}


def _get_nc():
    key = (M_LOCAL, N_TOTAL, C)
    if key not in _NC_CACHE:
        _NC_CACHE[key] = build_nc()
    return _NC_CACHE[key]


def run_cores(pred2d, tgt2d, trace=False):
    """Run the SPMD program on cores 0..7; returns (partials [8,128,MT], res)."""
    import ml_dtypes
    bf16 = ml_dtypes.bfloat16
    nc = _get_nc()
    pred_b = np.asarray(pred2d, dtype=np.float32).astype(bf16)
    tgt_b = np.asarray(tgt2d, dtype=np.float32).astype(bf16)
    in_maps = []
    for ci in range(N_CORES):
        r0 = ci * M_LOCAL
        in_maps.append({
            "pred": np.ascontiguousarray(pred_b[r0:r0 + M_LOCAL]),
            "tgt": tgt_b,
            "td": np.ascontiguousarray(tgt_b[r0:r0 + M_LOCAL]),
        })
    res = run_bass_kernel_spmd(nc, in_maps, list(range(N_CORES)), trace=trace)
    partials = np.stack([res.results[i]["out"] for i in range(N_CORES)])
    return partials, res


def kernel(pred, target):
    pred2d = np.asarray(pred, dtype=np.float32).reshape(-1, C)
    tgt2d = np.asarray(target, dtype=np.float32).reshape(-1, C)
    partials, _ = run_cores(pred2d, tgt2d)
    loss = partials.astype(np.float64).sum() / float(N_TOTAL)
    return np.float32(loss)
